# revision 31
# baseline (speedup 1.0000x reference)
"""Dense spatial self-attention block (LayerNorm + single-head attention +
residual) for Trainium2, data-parallel over batch across 8 NeuronCores.

Shapes (hardcoded from the problem spec):
  x: [B=8, H=64, W=64, C=256] fp32 -> out: same shape.
Each core processes one batch element: T = H*W = 4096 tokens, C = 256.

Algebraic folding (exact, softmax-invariance based):
  With hn = LayerNorm-normalized x (gamma/beta folded into the weights):
    q_i = hn_i Wqg + qb,  k_j = hn_j Wkg + kb
    S_ij = q_i.k_j  ==  (hn_i A + cvec) . hn_j   + per-row consts (cancel)
      where A = Wqg Wkg^T, cvec = Wkg qb            -> K projection eliminated
    out = softmax(S) V Wo + bvo  with V Wo = hn Wvo (Wvo = Wvg Wo)
      and bvo = (beta Wv + bv) Wo + bo              -> O projection eliminated
  So per 256-query block the PV accumulator directly holds the final
  attention output numerator plus a ones-column denominator.

fp8 attention (P.V in DoubleRow fp8, 2x PE throughput):
  exp is written by ScalarE directly as e5m2 (22 e-fold range: no softmax
  tail-mass loss; validated vs the 2e-2 gate at l2~1.7e-2), V as an e4m3
  hi+lo pair (single e4m3 V would breach the error budget).
  The exp offset M keeps P in e5m2's window; it cancels through the
  denominator so only its magnitude matters. M is computed on device from
  max_i ||q'_i|| (LN makes ||hn_j|| = 16 exactly, so sigma_i = ||q'_i||/16
  bounds the per-query logit scale): M = (GAMMA/16) sqrt(max ssq) - MOFF.
  S stays bf16 (fp8 S fails the error budget even split 2-ways).
"""

import math

import numpy as np

import concourse.bass as bass
import concourse.mybir as mybir
import concourse.tile as tile
from concourse.bass_utils import run_bass_kernel_spmd
from concourse.masks import make_identity

F32 = mybir.dt.float32
BF16 = mybir.dt.bfloat16
E4 = mybir.dt.float8e4
E5 = mybir.dt.float8e5
AF = mybir.ActivationFunctionType
OP = mybir.AluOpType
PM = mybir.MatmulPerfMode
AX = mybir.AxisListType

B, HH, WW, C = 8, 64, 64, 256
T = HH * WW            # 4096 tokens per core
P = 128
CT = C // P            # 2 channel planes
TT = T // P            # 32 token tiles
NB = T // 512          # 8 Q-projection slices of 512 tokens
TBLK = 256             # query-block size for attention
NTB = T // TBLK        # 16 query blocks
GRP = 4                # key chunks per exp group
NG = TT // GRP         # 8 groups per block
EPS = 1e-5
SCALE = float(C) ** -0.5
GAMMA = 4.6            # sigma multiplier for the exp-offset estimate
MOFF = 9.0             # centers P in e5m2's range
LNBIAS = math.log(GAMMA / 16.0)

MAX_WAITS_PER_INST = 1


def _split_multi_waits(nc: bass.Bass, max_waits: int = MAX_WAITS_PER_INST):
    """This container's walrus rejects instructions carrying more than ~1
    sync-wait ("Too many sync wait commands"). Hoist excess waits onto
    preceding same-engine InstNoOps (waiting earlier is always safe)."""
    n_split = 0
    for f in nc.m.functions:
        for bb in f.blocks:
            new_insts = []
            for inst in bb.instructions:
                si = getattr(inst, "sync_info", None)
                if si is not None and si.on_wait and len(si.on_wait) > max_waits:
                    waits = list(si.on_wait)
                    keep = waits[-max_waits:]
                    extra = waits[:-max_waits]
                    for i in range(0, len(extra), max_waits):
                        nop = mybir.InstNoOp(
                            name=nc.get_next_instruction_name(), ins=[], outs=[]
                        )
                        nop.engine = inst.engine
                        nop.sync_info = mybir.SyncInfo(
                            on_wait=extra[i : i + max_waits], on_update=[]
                        )
                        nc.register_instruction(nop, overwrite=True)
                        new_insts.append(nop)
                    si.on_wait = keep
                    n_split += 1
                new_insts.append(inst)
            bb.instructions[:] = new_insts
    return n_split


def build(n_reps: int = 1) -> bass.Bass:
    nc = bass.Bass()

    x_d = nc.declare_dram_parameter("x", [T, C], F32, isOutput=False)
    gamma_d = nc.declare_dram_parameter("ln_gamma", [C], F32, isOutput=False)
    beta_d = nc.declare_dram_parameter("ln_beta", [C], F32, isOutput=False)
    wq_d = nc.declare_dram_parameter("wq", [C, C], F32, isOutput=False)
    bq_d = nc.declare_dram_parameter("bq", [C], F32, isOutput=False)
    wk_d = nc.declare_dram_parameter("wk", [C, C], F32, isOutput=False)
    bk_d = nc.declare_dram_parameter("bk", [C], F32, isOutput=False)
    wv_d = nc.declare_dram_parameter("wv", [C, C], F32, isOutput=False)
    bv_d = nc.declare_dram_parameter("bv", [C], F32, isOutput=False)
    wo_d = nc.declare_dram_parameter("wo", [C, C], F32, isOutput=False)
    bo_d = nc.declare_dram_parameter("bo", [C], F32, isOutput=False)
    out_d = nc.declare_dram_parameter("out", [T, C], F32, isOutput=True)

    x_tiled = x_d.rearrange("(o p) c -> p o c", p=P)      # [128, 32, 256]
    out_tiled = out_d.rearrange("(o p) c -> p o c", p=P)  # [128, 32, 256]

    with tile.TileContext(nc) as tc:
        _body(tc, nc, x_tiled, out_tiled, gamma_d, beta_d,
              wq_d, bq_d, wk_d, bk_d, wv_d, bv_d, wo_d, bo_d, n_reps)
    _split_multi_waits(nc, MAX_WAITS_PER_INST)
    return nc


def _body(tc, nc, x_tiled, out_tiled, gamma_d, beta_d,
          wq_d, bq_d, wk_d, bk_d, wv_d, bv_d, wo_d, bo_d, n_reps):
    from contextlib import ExitStack

    ctx = ExitStack()
    singles = ctx.enter_context(tc.tile_pool(name="singles", bufs=1))
    temps = ctx.enter_context(tc.tile_pool(name="temps", bufs=3))
    stats_p = ctx.enter_context(tc.tile_pool(name="stats", bufs=4))
    # PSUM: s (2 banks x 2) + om0 (1 x 2) + om1 (1 x 2) = 8 banks exactly.
    ps_s = ctx.enter_context(tc.tile_pool(name="ps_s", bufs=2, space="PSUM"))
    ps_om0 = ctx.enter_context(tc.tile_pool(name="ps_om0", bufs=2, space="PSUM"))
    ps_om1 = ctx.enter_context(tc.tile_pool(name="ps_om1", bufs=2, space="PSUM"))

    # ---- constants -----------------------------------------------------
    ident = singles.tile([P, P], BF16)
    make_identity(nc, ident)

    gamma_col = singles.tile([P, CT], F32)
    nc.sync.dma_start(out=gamma_col, in_=gamma_d.rearrange("(o p) -> p o", p=P))
    beta_col = singles.tile([P, CT], F32)
    nc.sync.dma_start(out=beta_col, in_=beta_d.rearrange("(o p) -> p o", p=P))
    bq_col = singles.tile([P, CT], F32)
    nc.sync.dma_start(out=bq_col, in_=bq_d.rearrange("(o p) -> p o", p=P))
    bv_col = singles.tile([P, CT], F32)
    nc.sync.dma_start(out=bv_col, in_=bv_d.rearrange("(o p) -> p o", p=P))
    bo_row = singles.tile([1, C], F32)
    nc.sync.dma_start(out=bo_row, in_=bo_d[None, :])
    eps_t = singles.tile([P, 1], F32)
    nc.vector.memset(eps_t, EPS)
    lnbias_t = singles.tile([1, 1], F32)
    nc.vector.memset(lnbias_t, LNBIAS)
    ones_row = singles.tile([1, P], BF16)
    nc.vector.memset(ones_row, 1.0)
    ones_col = singles.tile([P, 1], BF16)
    nc.vector.memset(ones_col, 1.0)
    # Dummy Ln to pull the one-time ~2.7us natural_log_exp table load onto
    # ScalarE while the x DMA is still in flight.
    act_warm = singles.tile([P, 1], F32)
    nc.scalar.activation(out=act_warm, in_=eps_t, func=AF.Ln, bias=1.0)

    # ---- big SBUF tensors ----------------------------------------------
    x_sb = singles.tile([P, TT, C], F32)       # x, later x + bvo
    ht = singles.tile([P, CT, T], BF16)        # hn^T
    qt = singles.tile([P, CT, T], BF16)        # Q'^T
    vh = singles.tile([P, TT, C + 1], E4)      # V' hi + ones column
    vl = singles.tile([P, TT, C + 1], E4)      # V' lo + zeros column
    nc.vector.memset(vh[:, :, C : C + 1], 1.0)
    nc.vector.memset(vl[:, :, C : C + 1], 0.0)

    wq_stg = singles.tile([P, CT, C], F32)
    wk_stg = singles.tile([P, CT, C], F32)
    wv_stg = singles.tile([P, CT, C], F32)
    wo_stg = singles.tile([P, CT, C], F32)
    wqg = singles.tile([P, CT, C], BF16)
    wkg = singles.tile([P, CT, C], BF16)
    wvg = singles.tile([P, CT, C], BF16)
    wo_bf = singles.tile([P, CT, C], BF16)
    wqgT = singles.tile([P, CT, C], BF16)
    wkgT = singles.tile([P, CT, C], BF16)
    wvgT = singles.tile([P, CT, C], BF16)
    a_sb = singles.tile([P, CT, C], BF16)      # A = Wqg Wkg^T  (c part, c' free)
    wvo_sb = singles.tile([P, CT, C], BF16)    # Wvo = Wvg Wo   (c part, e free)
    cvec_col = singles.tile([P, CT], F32)
    qb_bf = singles.tile([P, CT], BF16)
    vb_bf = singles.tile([P, CT], BF16)
    bvo_bcast = singles.tile([P, C], F32)
    m_col = singles.tile([P, 1], F32)          # exp bias = -(M), blocks 2+
    m_col01 = singles.tile([P, 1], F32)        # early exp bias for blocks 0/1
    ssqmax = singles.tile([1, NB], F32)

    def emit_weight_dmas():
        nc.sync.dma_start(out=wq_stg, in_=wq_d.rearrange("(o p) d -> p o d", p=P))
        nc.sync.dma_start(out=wk_stg, in_=wk_d.rearrange("(o p) d -> p o d", p=P))
        nc.sync.dma_start(out=wv_stg, in_=wv_d.rearrange("(o p) d -> p o d", p=P))
        nc.sync.dma_start(out=wo_stg, in_=wo_d.rearrange("(o p) d -> p o d", p=P))

    def emit_weight_folds():
        # gamma-folded bf16 weights + plain wo
        for ct in range(CT):
            nc.vector.tensor_scalar_mul(wqg[:, ct], wq_stg[:, ct], gamma_col[:, ct : ct + 1])
            nc.vector.tensor_scalar_mul(wkg[:, ct], wk_stg[:, ct], gamma_col[:, ct : ct + 1])
            nc.vector.tensor_scalar_mul(wvg[:, ct], wv_stg[:, ct], gamma_col[:, ct : ct + 1])
            nc.vector.tensor_copy(wo_bf[:, ct], wo_stg[:, ct])
        # transposes: w[c, d] -> wT[d, c]
        for (w_sb, wT) in ((wqg, wqgT), (wkg, wkgT), (wvg, wvgT)):
            for oc in range(CT):
                tp = ps_om0.tile([P, CT, P], BF16, tag="om0", name="tp")
                for dt in range(CT):
                    nc.tensor.transpose(tp[:, dt], w_sb[:, oc, dt * P : (dt + 1) * P], ident)
                for dt in range(CT):
                    nc.vector.tensor_copy(wT[:, dt, oc * P : (oc + 1) * P], tp[:, dt])
        # A = Wqg Wkg^T ; Wvo = Wvg Wo
        for (lT, rhs_sb, dst) in ((wqgT, wkgT, a_sb), (wvgT, wo_bf, wvo_sb)):
            for ct in range(CT):
                ps = ps_om1.tile([P, C], F32, tag="om1", name="a_ps")
                for pl in range(CT):
                    nc.tensor.matmul(
                        ps,
                        lhsT=lT[:, pl, ct * P : (ct + 1) * P],
                        rhs=rhs_sb[:, pl, :],
                        start=(pl == 0),
                        stop=(pl == CT - 1),
                    )
                nc.vector.tensor_copy(dst[:, ct], ps)
        # qb = beta Wq + bq ; vb = beta Wv + bv   (columns, [d_part, dt])
        for (w_stg, b_col, dst) in ((wq_stg, bq_col, qb_bf), (wv_stg, bv_col, vb_bf)):
            for dt in range(CT):
                ps = ps_om0.tile([P, 1], F32, tag="om0", name="qb_ps")
                for oc in range(CT):
                    nc.tensor.matmul(
                        ps,
                        lhsT=w_stg[:, oc, dt * P : (dt + 1) * P],
                        rhs=beta_col[:, oc : oc + 1],
                        start=(oc == 0),
                        stop=(oc == CT - 1),
                    )
                nc.vector.tensor_tensor(
                    out=dst[:, dt : dt + 1], in0=ps, in1=b_col[:, dt : dt + 1], op=OP.add
                )
        # cvec = Wkg qb   (column, [c'_part, ct])
        for ct in range(CT):
            ps = ps_om0.tile([P, 1], F32, tag="om0", name="cv_ps")
            for pl in range(CT):
                nc.tensor.matmul(
                    ps,
                    lhsT=wkgT[:, pl, ct * P : (ct + 1) * P],
                    rhs=qb_bf[:, pl : pl + 1],
                    start=(pl == 0),
                    stop=(pl == CT - 1),
                )
            nc.vector.tensor_copy(cvec_col[:, ct : ct + 1], ps)
        # bvo = vb Wo + bo ; broadcast down partitions
        psb = ps_om0.tile([1, C], F32, tag="om0", name="bvo_ps")
        for pl in range(CT):
            nc.tensor.matmul(
                psb,
                lhsT=vb_bf[:, pl : pl + 1],
                rhs=wo_bf[:, pl, :],
                start=(pl == 0),
                stop=(pl == CT - 1),
            )
        bvo_bf = singles.tile([1, C], BF16)
        nc.vector.tensor_tensor(out=bvo_bf, in0=psb, in1=bo_row, op=OP.add)
        psbc = ps_om1.tile([P, C], F32, tag="om1", name="bvo_bc")
        nc.tensor.matmul(psbc, lhsT=ones_row, rhs=bvo_bf, start=True, stop=True)
        nc.vector.tensor_copy(bvo_bcast, psbc)

    # ---- LayerNorm machinery (batched stats as in the bf16 baseline) ---
    LNG = 8
    LN_GROUPS = [4, 4, 8, 8, 8]

    def emit_ln_stats(tt, mv_all, col):
        stats = stats_p.tile([P, 6], F32, name="stats")
        nc.vector.bn_stats(out=stats, in_=x_sb[:, tt, :])
        nc.vector.bn_aggr(out=mv_all[:, col], in_=stats)

    def emit_ln_rsqrt(mv_all, n):
        # rstd = exp(-0.5*ln(var+eps)) keeps ScalarE inside the
        # natural_log_exp table set (no ~2.7us table reload).
        v = mv_all[:, :n, 1]
        nc.scalar.activation(out=v, in_=v, func=AF.Ln, bias=eps_t)
        nc.scalar.activation(out=v, in_=v, func=AF.Exp, scale=-0.5)

    def emit_ln_apply(tt, mv_all, col, add_resid=True):
        xt = x_sb[:, tt, :]
        h_bf = temps.tile([P, C], BF16, name="h_bf")
        nc.vector.tensor_scalar(
            out=h_bf, in0=xt,
            scalar1=mv_all[:, col, 0:1], scalar2=mv_all[:, col, 1:2],
            op0=OP.subtract, op1=OP.mult,
        )
        tp = ps_s.tile([P, CT, P], BF16, tag="s", name="tph")
        for ct in range(CT):
            nc.tensor.transpose(tp[:, ct], h_bf[:, ct * P : (ct + 1) * P], ident)
        nc.vector.tensor_copy(out=ht[:, :, tt * P : (tt + 1) * P], in_=tp)
        # x_sb <- x + bvo (residual incl. folded out-proj bias), after LN reads
        if add_resid:
            nc.gpsimd.tensor_add(out=xt, in0=xt, in1=bvo_bcast)

    def emit_v_pair(jt0):
        psu = ps_s.tile([P, 2, C], F32, tag="s", name="v_ps")
        for jj in range(2):
            for ct in range(CT):
                nc.tensor.matmul(
                    psu[:, jj],
                    lhsT=ht[:, ct, (jt0 + jj) * P : (jt0 + jj + 1) * P],
                    rhs=wvo_sb[:, ct, :],
                    start=(ct == 0),
                    stop=(ct == CT - 1),
                )
        nc.vector.tensor_copy(vh[:, jt0 : jt0 + 2, 0:C], psu)
        # (gpsimd cannot read PSUM, so the lo-residual stays on DVE)
        nc.vector.tensor_tensor(
            out=vl[:, jt0 : jt0 + 2, 0:C],
            in0=psu,
            in1=vh[:, jt0 : jt0 + 2, 0:C],
            op=OP.subtract,
        )

    def emit_q_proj(ntv):
        for dt in range(CT):
            ps = ps_s.tile([P, 512], F32, tag="s", name="q_ps")
            for ct in range(CT):
                nc.tensor.matmul(
                    ps,
                    lhsT=a_sb[:, ct, dt * P : (dt + 1) * P],
                    rhs=ht[:, ct, ntv * 512 : (ntv + 1) * 512],
                    start=(ct == 0),
                    stop=(ct == CT - 1),
                )
            nc.vector.tensor_scalar(
                out=qt[:, dt, ntv * 512 : (ntv + 1) * 512],
                in0=ps,
                scalar1=cvec_col[:, dt : dt + 1], scalar2=None,
                op0=OP.add,
            )

    def emit_ssq(ntv):
        sq = temps.tile([P, CT, 512], BF16, name="sq")
        qs = qt[:, :, ntv * 512 : (ntv + 1) * 512]
        nc.gpsimd.tensor_tensor(out=sq, in0=qs, in1=qs, op=OP.mult)
        ps = ps_s.tile([1, 512], F32, tag="s", name="ssq_ps")
        for pl in range(CT):
            nc.tensor.matmul(
                ps, lhsT=ones_col, rhs=sq[:, pl], start=(pl == 0), stop=(pl == CT - 1)
            )
        nc.vector.tensor_reduce(ssqmax[:, ntv : ntv + 1], ps, AX.X, OP.max)

    def emit_m_calc(src, dst):
        # M = (GAMMA/16) sqrt(max ssq) - MOFF ; dst = -M broadcast.
        # src selects the ssqmax slice range: per-block-group offsets are
        # exact (they cancel through the softmax denominator), so woven
        # blocks 0/1 use an early M from Q-slice 0 alone.
        mm = stats_p.tile([1, 1], F32, name="mm")
        nc.vector.tensor_reduce(mm, src, AX.X, OP.max)
        nc.scalar.activation(out=mm, in_=mm, func=AF.Ln, bias=0.0)
        nc.scalar.activation(out=mm, in_=mm, func=AF.Exp, scale=0.5, bias=lnbias_t)
        m_bf = stats_p.tile([1, 1], BF16, name="m_bf")
        nc.vector.tensor_scalar(
            out=m_bf, in0=mm, scalar1=-1.0, scalar2=MOFF, op0=OP.mult, op1=OP.add
        )
        psm = ps_s.tile([P, 1], F32, tag="s", name="m_ps")
        nc.tensor.matmul(psm, lhsT=ones_row, rhs=m_bf, start=True, stop=True)
        nc.vector.tensor_copy(dst, psm)

    # ---- attention -----------------------------------------------------
    def emit_s_group(tb, g):
        s_ps = ps_s.tile([P, GRP, TBLK], F32, tag="s", name="s_ps")
        for j in range(GRP):
            jc = g * GRP + j
            for ct in range(CT):
                nc.tensor.matmul(
                    s_ps[:, j],
                    lhsT=ht[:, ct, jc * P : (jc + 1) * P],
                    rhs=qt[:, ct, tb * TBLK : (tb + 1) * TBLK],
                    start=(ct == 0),
                    stop=(ct == CT - 1),
                )
        return s_ps

    def emit_exp_pv(tb, g, s_ps, o_tiles, bias_col):
        pt = temps.tile([P, GRP, TBLK], E5, name="pt")
        nc.scalar.activation(
            out=pt.rearrange("p a b -> p (a b)"),
            in_=s_ps.rearrange("p a b -> p (a b)"),
            func=AF.Exp, scale=SCALE, bias=bias_col,
        )
        for pair in range(GRP // 2):
            j0 = g * GRP + 2 * pair
            for m in range(2):
                lhsT = pt[:, 2 * pair : 2 * pair + 2, m * P : (m + 1) * P]
                for vi, v_sb in enumerate((vh, vl)):
                    nc.tensor.matmul(
                        o_tiles[m],
                        lhsT=lhsT,
                        rhs=v_sb[:, j0 : j0 + 2, 0 : C + 1],
                        start=(g == 0 and pair == 0 and vi == 0),
                        stop=(g == NG - 1 and pair == 1 and vi == 1),
                        perf_mode=PM.DoubleRow,
                    )

    def block_groups(tb, o_tiles, bias_col):
        # generator: one yield per S-group emission (exp/PV lag one group)
        s_prev = None
        for g in range(NG):
            s_cur = emit_s_group(tb, g)
            if g >= 1:
                emit_exp_pv(tb, g - 1, s_prev, o_tiles, bias_col)
            s_prev = s_cur
            yield g
        emit_exp_pv(tb, NG - 1, s_prev, o_tiles, bias_col)
        yield NG

    def emit_epilogue(tb, o_tiles):
        for m in range(2):
            gt = 2 * tb + m
            rec = stats_p.tile([P, 1], F32, name="rec")
            nc.vector.reciprocal(rec, o_tiles[m][:, C : C + 1])
            oo = temps.tile([P, C], F32, name="oo")
            nc.vector.tensor_scalar_mul(oo, o_tiles[m][:, 0:C], rec)
            nc.gpsimd.tensor_add(out=oo, in0=oo, in1=x_sb[:, gt, :])
            nc.sync.dma_start(out=out_tiled[:, gt, :], in_=oo)

    for rep in range(n_reps):
        # ---- x loads: first chunks first so LN starts early ------------
        nc.sync.dma_start(out=x_sb[:, 0:2, :], in_=x_tiled[:, 0:2, :])
        nc.sync.dma_start(out=x_sb[:, 2:4, :], in_=x_tiled[:, 2:4, :])
        if rep == 0:
            emit_weight_dmas()
        for g in range(1, 8):
            nc.sync.dma_start(
                out=x_sb[:, g * 4 : (g + 1) * 4, :],
                in_=x_tiled[:, g * 4 : (g + 1) * 4, :],
            )

        mv_first = stats_p.tile([P, LNG, 2], F32, name="mv_all")
        for i in range(LN_GROUPS[0]):
            emit_ln_stats(i, mv_first, col=i)
        emit_ln_rsqrt(mv_first, LN_GROUPS[0])

        tt_base = 0
        for g, gsz in enumerate(LN_GROUPS):
            if g == 0:
                mv_all = mv_first
            else:
                mv_all = stats_p.tile([P, LNG, 2], F32, name="mv_all")
                for i in range(gsz):
                    emit_ln_stats(tt_base + i, mv_all, col=i)
                emit_ln_rsqrt(mv_all, gsz)
            for i in range(gsz):
                tt = tt_base + i
                # bvo_bcast is produced by the weight folds (at tt==1); the
                # residual adds for tiles 0..1 are deferred until after it.
                emit_ln_apply(tt, mv_all, i, add_resid=(tt >= 2 or rep > 0))
                if tt == 1 and rep == 0:
                    emit_weight_folds()
                    for t_early in (0, 1):
                        nc.gpsimd.tensor_add(
                            out=x_sb[:, t_early, :],
                            in0=x_sb[:, t_early, :],
                            in1=bvo_bcast,
                        )
                if tt % 2 == 1:
                    emit_v_pair(tt - 1)
                if tt % 4 == 3:
                    emit_q_proj(tt // 4)
                    emit_ssq(tt // 4)
                if tt == 3:
                    # early exp offset for the woven blocks 0/1 (their
                    # queries live entirely in Q-slice 0)
                    emit_m_calc(ssqmax[:, 0:1], m_col01)
                    o_w = [
                        [ps_om0.tile([P, C + 1], F32, tag="om0", name="o0"),
                         ps_om1.tile([P, C + 1], F32, tag="om1", name="o1")]
                        for _ in range(2)
                    ]
                    gens = [block_groups(tb_w, o_w[tb_w], m_col01)
                            for tb_w in range(2)]
                    steps = [0, 0]
                if tt >= 4:
                    # weave blocks 0/1 into the LN phase: group g needs ht
                    # and V chunks 4g..4g+3, available once tile 4g+3 is done
                    allowed = min((tt + 1) // 4, NG)
                    budget = 2
                    for gi in range(2):
                        while budget > 0 and steps[gi] < allowed:
                            next(gens[gi])
                            steps[gi] += 1
                            budget -= 1
            tt_base += gsz

        # finish the woven blocks (their last chunks arrive only at LN end)
        for gi in range(2):
            while steps[gi] <= NG:
                next(gens[gi], None)
                steps[gi] += 1
        emit_m_calc(ssqmax, m_col)
        emit_epilogue(0, o_w[0])
        emit_epilogue(1, o_w[1])

        # ---- remaining attention blocks --------------------------------
        for tb in range(2, NTB):
            o_tiles = [
                ps_om0.tile([P, C + 1], F32, tag="om0", name="o0"),
                ps_om1.tile([P, C + 1], F32, tag="om1", name="o1"),
            ]
            s_prev = None
            for g in range(NG):
                s_cur = emit_s_group(tb, g)
                if g >= 1:
                    emit_exp_pv(tb, g - 1, s_prev, o_tiles, m_col)
                s_prev = s_cur
            emit_exp_pv(tb, NG - 1, s_prev, o_tiles, m_col)
            emit_epilogue(tb, o_tiles)

    ctx.close()


_cache = {}


def _get_nc(n_reps: int = 1):
    if n_reps not in _cache:
        _cache[n_reps] = build(n_reps)
    return _cache[n_reps]


def _make_in_maps(inputs):
    x = np.ascontiguousarray(np.asarray(inputs["x"], dtype=np.float32))
    shared = {
        k: np.ascontiguousarray(np.asarray(inputs[k], dtype=np.float32))
        for k in ("ln_gamma", "ln_beta", "wq", "bq", "wk", "bk", "wv", "bv", "wo", "bo")
    }
    return [dict(shared, x=x[i].reshape(T, C)) for i in range(B)]


def kernel(**inputs: np.ndarray) -> np.ndarray:
    nc = _get_nc(1)
    in_maps = _make_in_maps(inputs)
    res = run_bass_kernel_spmd(nc, in_maps, list(range(B)))
    out = np.stack(
        [res.results[i]["out"].reshape(HH, WW, C) for i in range(B)], axis=0
    )
    return out.astype(np.float32)
